# revision 48
# baseline (speedup 1.0000x reference)
"""Trainium2 Bass kernel for a cached-attention decode step (B=16, T=1, D=4096,
H=32, DK=128, S=2048), tensor-parallel over heads across 8 NeuronCores.

Sharding: each core owns 4 heads: column-sharded Wq/Wk/Wv (512 rows each),
the matching slices of the KV cache, and the matching 512 columns of Wo.

v3 design (vs the 196us baseline):
  - KV cache streamed entirely as fp8 e3m4 (33.5 MB/core); Wk/Wv fp8 x64
    (their quantization only touches the single new token); Wq/Wo fp16.
  - No k_new scatter into the cache: the host zeroes K column start_pos, the
    kernel exports pl = exp(q.k_new - SHIFT) and the host corrects
    Z = z - exp(-SHIFT) + pl. This removes the proj->attention serial
    dependency: scores depend only on q^T.
  - Monolithic weight DMAs (one transfer per tensor) to amortize the
    ~0.6us/transfer HWDGE fixed cost.
  - q-projection chains run first so attention starts ~13us into the body
    while wkv/wo still stream; y-projection runs inside the head loop.
  - kv is alone on the sync HWDGE ring (no head-of-line blocking); weights,
    x and all small stores ride the scalar/ACT ring.
  - Projections are weight-stationary matmuls producing q^T/k^T/v^T directly
    in [d, (h,b)] column layout (no PE transposes). All chains of a PSUM
    bank are opened by one zero matmul (first_mm clears the whole bank).

Math per core, per local head h and batch b:
    P = exp(scores/sqrt(DK) - EXP_SHIFT); zsum accumulated by the ACT engine
    AO_unnorm = P @ V (cache rows; the stale V row at start_pos is excluded
                       via the r_new row split) + pl * v_new
    y_h = AO_unnorm^T @ Wo_slice   (per-head, unnormalized)
Host divides y_h by Z per (head, batch), sums over heads and cores.
"""

from contextlib import ExitStack

import numpy as np

import concourse.bacc as bacc
import concourse.mybir as mybir
import concourse.tile as tile
from concourse.bass_utils import run_bass_kernel_spmd

B = 16          # batch
H = 32          # total heads
D = 4096        # model dim
DK = 128        # head dim
NCORES = 8
HL = H // NCORES            # 4 local heads per core
FL = HL * DK                # 512 local features per core
KT = D // 128               # 32 contraction tiles over D
F32 = mybir.dt.float32
F16 = mybir.dt.float16
F8 = mybir.dt.float8e3
AF = mybir.ActivationFunctionType

_PROGRAM_CACHE: dict = {}
_VARIANT = "full"  # "full" | "dma_only" | "compute_only" | "attn_only"
EXP_SHIFT = 2.0    # P = exp(score - EXP_SHIFT): cancels in host Z-normalization
SCALE = DK ** -0.5
WKV_SCALE = 64.0   # Wk/Wv pre-scaled x64 on host so fp8 e3m4 stays in range


def build_program(S_eff: int, repeat: int = 1, G: int = 2, kv_bufs: int = 8,
                  sc_bufs: int = 2, p_bufs: int = 4):
    """Emit the per-core Bass/Tile program (identical across all cores)."""
    NT = (S_eff + 127) // 128   # s-tiles incl. the partially-valid last tile
    S_pad = NT * 128
    r_new = (S_eff - 1) % 128   # row of the new token within the last s-tile
    SPV = S_pad + NT * DK       # fused fp8 K^T + V slab width per (h, b)
    assert B % G == 0
    HB = HL * B

    nc = bacc.Bacc("TRN2", num_devices=NCORES)
    xt = nc.declare_dram_parameter("xt", [128, KT, B], F16, isOutput=False)
    # wq is head-major so head h's q chain can start after 1/HL of the bytes
    wq_d = nc.declare_dram_parameter("wq", [128, HL, KT, DK], F16, isOutput=False)
    wkv_d = nc.declare_dram_parameter("wkv", [128, KT, 2 * FL], F8, isOutput=False)
    wo_d = nc.declare_dram_parameter("wo", [128, HL, D], F16, isOutput=False)
    kv_d = nc.declare_dram_parameter(
        "kv", [HL, B // G, 128, G, SPV], F8, isOutput=False
    )
    y_d = nc.declare_dram_parameter("y", [HL, B, D], F16, isOutput=True)
    z_d = nc.declare_dram_parameter("z", [1, HB], F32, isOutput=True)
    pl_d = nc.declare_dram_parameter("pl", [1, HB], F32, isOutput=True)

    with tile.TileContext(nc) as tc, ExitStack() as ctx:
        singles = ctx.enter_context(tc.tile_pool(name="singles", bufs=1))
        kvpool = ctx.enter_context(tc.tile_pool(name="kvp", bufs=kv_bufs))
        ppool = ctx.enter_context(tc.tile_pool(name="ppool", bufs=p_bufs))
        ysbp = ctx.enter_context(tc.tile_pool(name="ysbp", bufs=2))
        pqps = ctx.enter_context(tc.tile_pool(name="pq_ps", bufs=2, space="PSUM"))
        pkvps = ctx.enter_context(tc.tile_pool(name="pkv_ps", bufs=1, space="PSUM"))
        scps = ctx.enter_context(tc.tile_pool(name="sc_ps", bufs=sc_bufs, space="PSUM"))
        aops = ctx.enter_context(tc.tile_pool(name="ao_ps", bufs=1, space="PSUM"))
        mips = ctx.enter_context(tc.tile_pool(name="misc_ps", bufs=2, space="PSUM"))

        ones_col = singles.tile([128, 1], F32)
        nc.vector.memset(ones_col, 1.0)
        ones_row = singles.tile([1, 128], F32)
        nc.vector.memset(ones_row, 1.0)
        zero_q = singles.tile([1, HB], F32)
        nc.vector.memset(zero_q, 0.0)
        zero_kv = singles.tile([1, 2 * HB], F32)
        nc.vector.memset(zero_kv, 0.0)
        nbias = singles.tile([128, 1], F32)     # exp bias (-EXP_SHIFT)
        nc.vector.memset(nbias, -EXP_SHIFT)

        wq_sb = singles.tile([128, HL, KT, DK], F16)
        wkv_sb = singles.tile([128, KT, 2 * FL], F8)
        wo_sb = singles.tile([128, HL, D], F16)
        xt_sb = singles.tile([128, KT, B], F16)
        qT_sb = singles.tile([128, HB], F16)    # q^T columns per (h, b)
        if _VARIANT == "attn_only":
            nc.vector.memset(qT_sb, 0.01)
        kT_sb = singles.tile([128, HB], F32)    # k_new^T (true scale)
        vn_sb = singles.tile([128, HB], F32)    # v_new^T (true scale)
        vpl_sb = singles.tile([128, HB], F32)   # pl * v_new^T correction
        tmp_sb = singles.tile([128, HB], F32)
        pl_sb = singles.tile([1, HB], F32)      # exp(q . k_new - SHIFT)
        zsum_sb = singles.tile([128, HB], F32)
        ao_sb = singles.tile([128, HB], F16)
        z_sb = singles.tile([1, HB], F32)
        nc.vector.memset(z_sb, 0.0)
        nc.vector.memset(pl_sb, 0.0)
        if _VARIANT == "compute_only":
            kv_fix = singles.tile([128, SPV], F8)
            nc.vector.memset(kv_fix, 0.01)

        def body():
            dma_only = _VARIANT == "dma_only"
            attn_only = _VARIANT == "attn_only"

            if attn_only:
                for h in range(HL):
                    for g in range(B // G):
                        kv_grp = kvpool.tile([128, G, SPV], F8, tag="kv")
                        nc.sync.dma_start(out=kv_grp, in_=kv_d[h, g])
                        for j in range(G):
                            b = g * G + j
                            col = h * B + b
                            kv_sb = kv_grp[:, j, :]
                            sc_ps = scps.tile([128, NT], F32, tag="sc")
                            for t in range(NT):
                                nc.tensor.matmul(
                                    sc_ps[:, t:t + 1],
                                    lhsT=kv_sb[:, t * 128:(t + 1) * 128],
                                    rhs=qT_sb[:, col:col + 1],
                                    start=True, stop=True,
                                )
                            p_sb = ppool.tile([128, NT], F16, tag="p")
                            nc.scalar.activation(
                                out=p_sb, in_=sc_ps, func=AF.Exp,
                                bias=nbias[:128], scale=SCALE,
                                accum_out=zsum_sb[:, col:col + 1],
                            )
                            ao_ps = aops.tile([DK, B], F32, tag="ao")
                            for t in range(NT):
                                nc.tensor.matmul(
                                    ao_ps[:, b:b + 1],
                                    lhsT=kv_sb[:, S_pad + t * DK:S_pad + (t + 1) * DK],
                                    rhs=p_sb[:, t:t + 1],
                                    start=t == 0, stop=t == NT - 1,
                                )
                    nc.vector.tensor_copy(ao_sb[:, h * B:(h + 1) * B], ao_ps)
                z_ps = mips.tile([1, HB], F32, tag="mi")
                nc.tensor.matmul(z_ps, lhsT=ones_col, rhs=zsum_sb,
                                 start=True, stop=True)
                nc.vector.tensor_copy(z_sb, z_ps)
                nc.sync.dma_start(out=z_d[:, :], in_=z_sb)
                nc.scalar.dma_start(out=pl_d[:, :], in_=pl_sb)
                return

            # ---- loads: few big transfers, kv alone on the sync ring ----
            # weights/x/outputs ride the GPSIMD SWDGE queues: the ACT
            # engine track must stay free for the 64 exps (a DMA issued
            # from nc.scalar occupies the ACT queue and delays every exp
            # behind it, stalling PV and the kv buffer recycling)
            nc.gpsimd.dma_start(out=xt_sb, in_=xt[:, :, :])
            for h in range(HL):
                nc.gpsimd.dma_start(out=wq_sb[:, h], in_=wq_d[:, h])
            nc.gpsimd.dma_start(out=wkv_sb, in_=wkv_d[:, :, :])
            nc.gpsimd.dma_start(out=wo_sb, in_=wo_d[:, :, :])
            if dma_only:
                for h in range(HL):
                    for g in range(B // G):
                        kv_grp = kvpool.tile([128, G, SPV], F8, tag="kv",
                                             name="kv_grp")
                        nc.sync.dma_start(out=kv_grp, in_=kv_d[h, g])
                nc.sync.dma_start(out=z_d[:, :], in_=z_sb)
                nc.scalar.dma_start(out=pl_d[:, :], in_=pl_sb)
                return

            # ---- q-projection chains first (attention depends only on q^T).
            # Two PSUM banks, two heads each: head h's chain completes right
            # after its 1MB wq slab lands, and the bank's qT copy doesn't
            # wait on the other bank's chains (PE-W/DVE-R same-bank hazard).
            for hb in range(2):
                q_ps = pqps.tile([128, 2 * B], F32, tag="pq", name=f"q_ps{hb}")
                nc.tensor.matmul(q_ps, lhsT=ones_row, rhs=zero_q[:, :2 * B],
                                 start=True, stop=False)
                for hh in range(2):
                    h = 2 * hb + hh
                    for kt in range(KT):
                        nc.tensor.matmul(
                            q_ps[:, hh * B:(hh + 1) * B],
                            lhsT=wq_sb[:, h, kt, :],
                            rhs=xt_sb[:, kt, :],
                            start=False, stop=kt == KT - 1,
                        )
                nc.vector.tensor_copy(qT_sb[:, 2 * hb * B:(2 * hb + 2) * B],
                                      q_ps)

            def emit_kv_chains():
                # k/v-projection chains (separate PSUM bank). Emitted AFTER
                # head 0's score/PV stream so the PE FIFO never stalls
                # waiting for the wkv DMA while kv groups are ready.
                kv_ps = pkvps.tile([128, 2 * HB], F32, tag="pkv",
                                   name="kv_ps")
                nc.tensor.matmul(kv_ps, lhsT=ones_row, rhs=zero_kv,
                                 start=True, stop=False)
                for kt in range(KT):
                    for h in range(HL):
                        nc.tensor.matmul(
                            kv_ps[:, h * B:(h + 1) * B],
                            lhsT=wkv_sb[:, kt, h * DK:(h + 1) * DK],
                            rhs=xt_sb[:, kt, :],
                            start=False, stop=kt == KT - 1,
                        )
                        nc.tensor.matmul(
                            kv_ps[:, HB + h * B:HB + (h + 1) * B],
                            lhsT=wkv_sb[:, kt, FL + h * DK:FL + (h + 1) * DK],
                            rhs=xt_sb[:, kt, :],
                            start=False, stop=kt == KT - 1,
                        )
                nc.scalar.activation(out=kT_sb, in_=kv_ps[:, :HB],
                                     func=AF.Copy, scale=1.0 / WKV_SCALE)
                nc.scalar.activation(out=vn_sb, in_=kv_ps[:, HB:],
                                     func=AF.Copy, scale=1.0 / WKV_SCALE)
                # pl = exp(q . k_new - SHIFT) per (h, b); vpl = pl * v_new
                nc.vector.tensor_mul(tmp_sb, qT_sb, kT_sb)
                pl_ps = mips.tile([1, HB], F32, tag="mi", name="pl_ps")
                nc.tensor.matmul(pl_ps, lhsT=ones_col, rhs=tmp_sb,
                                 start=True, stop=True)
                nc.scalar.activation(out=pl_sb, in_=pl_ps, func=AF.Exp,
                                     bias=nbias[:1], scale=SCALE)
                plb_ps = mips.tile([128, HB], F32, tag="mi", name="plb_ps")
                nc.tensor.matmul(plb_ps, lhsT=ones_row, rhs=pl_sb,
                                 start=True, stop=True)
                nc.vector.tensor_mul(vpl_sb, vn_sb, plb_ps)

            emit_kv_chains()

            # ---- attention over the cache, head by head ----
            for h in range(HL):
                ao_ps = aops.tile([DK, B], F32, tag="ao")
                pending = []  # software-pipeline PV one bh behind scores

                def emit_pv(ent):
                    b_, p_sb_, kv_sb_ = ent
                    n_full = NT - 1
                    for t in range(n_full):
                        nc.tensor.matmul(
                            ao_ps[:, b_:b_ + 1],
                            lhsT=kv_sb_[:, S_pad + t * DK:S_pad + (t + 1) * DK],
                            rhs=p_sb_[:, t:t + 1],
                            start=t == 0,
                            stop=t == n_full - 1 and r_new == 0,
                        )
                    if r_new > 0:
                        nc.tensor.matmul(
                            ao_ps[:, b_:b_ + 1],
                            lhsT=kv_sb_[:r_new, S_pad + (NT - 1) * DK:S_pad + NT * DK],
                            rhs=p_sb_[:r_new, NT - 1:NT],
                            start=n_full == 0, stop=True,
                        )

                for g in range(B // G):
                    if _VARIANT == "compute_only":
                        kv_grp = None
                    else:
                        kv_grp = kvpool.tile([128, G, SPV], F8, tag="kv",
                                             name="kv_grp")
                        nc.sync.dma_start(out=kv_grp, in_=kv_d[h, g])
                    for j in range(G):
                        b = g * G + j
                        col = h * B + b
                        kv_sb = kv_fix if kv_grp is None else kv_grp[:, j, :]
                        sc_ps = scps.tile([128, NT], F32, tag="sc")
                        for t in range(NT):
                            nc.tensor.matmul(
                                sc_ps[:, t:t + 1],
                                lhsT=kv_sb[:, t * 128:(t + 1) * 128],
                                rhs=qT_sb[:, col:col + 1],
                                start=True, stop=True,
                            )
                        p_sb = ppool.tile([128, NT], F16, tag="p")
                        nc.scalar.activation(
                            out=p_sb, in_=sc_ps, func=AF.Exp,
                            bias=nbias[:128], scale=SCALE,
                            accum_out=zsum_sb[:, col:col + 1],
                        )
                        pending.append((b, p_sb, kv_sb))
                        if len(pending) == 3:
                            emit_pv(pending.pop(0))
                while pending:
                    emit_pv(pending.pop(0))

                # correction add during PSUM -> SBUF copy
                nc.vector.tensor_add(
                    ao_sb[:, h * B:(h + 1) * B], ao_ps,
                    vpl_sb[:, h * B:(h + 1) * B],
                )
                # per-head output projection (unnormalized), y on scalar ring
                y_sb = ysbp.tile([B, D], F16, tag="ysb")
                for oc in range(D // 512):
                    y_ps = mips.tile([B, 512], F32, tag="mi")
                    nc.tensor.matmul(
                        y_ps,
                        lhsT=ao_sb[:, h * B:(h + 1) * B],
                        rhs=wo_sb[:, h, oc * 512:(oc + 1) * 512],
                        start=True, stop=True,
                    )
                    nc.vector.tensor_copy(y_sb[:, oc * 512:(oc + 1) * 512], y_ps)
                nc.gpsimd.dma_start(out=y_d[h], in_=y_sb)

            # Z per (h, b): sum zsum over partitions via ones-matmul
            z_ps = mips.tile([1, HB], F32, tag="mi")
            nc.tensor.matmul(z_ps, lhsT=ones_col, rhs=zsum_sb,
                             start=True, stop=True)
            nc.vector.tensor_copy(z_sb, z_ps)
            nc.gpsimd.dma_start(out=z_d[:, :], in_=z_sb)
            nc.gpsimd.dma_start(out=pl_d[:, :], in_=pl_sb)

        if repeat == 1:
            body()
        else:
            # PE body is ~2500 instructions (>one IRAM block): hint the
            # back-edge so the branch target prefetches (~3-4us/iter saved)
            with tc.For_i(0, repeat, 1, hint_engines=(mybir.EngineType.PE,)):
                body()

    nc.compile()
    return nc


def _prep_inputs(x, k_cache, v_cache, Wq, Wk, Wv, Wo, S_eff, G=2):
    """Host-side sharding + layout prep. Returns per-core input dicts."""
    import ml_dtypes

    NT = (S_eff + 127) // 128
    S_pad = NT * 128
    SPV = S_pad + NT * DK
    f8 = ml_dtypes.float8_e3m4

    x2 = np.asarray(x, dtype=np.float32).reshape(B, D)
    xt_tiled = np.ascontiguousarray(
        x2.T.reshape(KT, 128, B).transpose(1, 0, 2).astype(np.float16)
    )  # [128, KT, B]

    k_cache = np.asarray(k_cache, dtype=np.float32)
    v_cache = np.asarray(v_cache, dtype=np.float32)

    # fused fp8 slab: K^T cols [0, S_pad), V tiles [S_pad, S_pad + NT*DK).
    # K column S_eff-1 (the new token) is left ZERO: its score is then 0 and
    # contributes exp(-EXP_SHIFT) to zsum, corrected on the host with pl.
    kv_all = np.zeros((H, B, 128, SPV), dtype=f8)
    kv_all[:, :, :, :S_eff - 1] = (
        k_cache[:, :, :S_eff - 1, :].transpose(1, 0, 3, 2)
    )
    v_src = np.zeros((H, B, NT * 128, DK), dtype=f8)
    v_src[:, :, :S_eff] = v_cache[:, :, :S_eff].transpose(1, 0, 2, 3)
    kv_all[:, :, :, S_pad:] = (
        v_src.reshape(H, B, NT, 128, DK)
        .transpose(0, 1, 3, 2, 4)
        .reshape(H, B, 128, NT * DK)
    )
    del v_src

    Wq = np.asarray(Wq, dtype=np.float32)
    Wk = np.asarray(Wk, dtype=np.float32)
    Wv = np.asarray(Wv, dtype=np.float32)
    Wo = np.asarray(Wo, dtype=np.float32)

    in_maps = []
    for c in range(NCORES):
        rows = slice(c * FL, (c + 1) * FL)
        # head-major: [128, HL, KT, DK], element [p, h, kt, dk] =
        # Wq^T[kt*128+p, h*DK+dk] restricted to this core's rows
        wq_tiled = np.ascontiguousarray(
            Wq[rows].T.reshape(KT, 128, HL, DK)
            .transpose(1, 2, 0, 3).astype(np.float16)
        )
        wkv_c = np.concatenate(
            [Wk[rows].T * WKV_SCALE, Wv[rows].T * WKV_SCALE], axis=1
        )  # (D, 2*FL)
        wkv_tiled = np.ascontiguousarray(
            wkv_c.reshape(KT, 128, 2 * FL).transpose(1, 0, 2).astype(f8)
        )
        wo_c = Wo[:, rows].T  # (FL, D)
        wo_tiled = np.ascontiguousarray(
            wo_c.reshape(HL, 128, D).transpose(1, 0, 2).astype(np.float16)
        )
        # kv: [HL, B, 128, SPV] -> grouped contiguous [HL, B//G, 128, G, SPV]
        kv_c = kv_all[c * HL:(c + 1) * HL]
        kv_c = np.ascontiguousarray(
            kv_c.reshape(HL, B // G, G, 128, SPV).transpose(0, 1, 3, 2, 4)
        )
        in_maps.append({
            "xt": xt_tiled,
            "wq": wq_tiled,
            "wkv": wkv_tiled,
            "wo": wo_tiled,
            "kv": kv_c,
        })
    return in_maps


def _combine(results, S_eff):
    """Host-side unshard: divide per-head partials by Z, sum everything."""
    NT = (S_eff + 127) // 128
    # zero cache columns (padding + the zeroed new-token column) each
    # contribute exp(0 - EXP_SHIFT) to zsum; the true new-token term is pl.
    pad_z = (NT * 128 - S_eff + 1) * np.exp(-EXP_SHIFT)
    y = np.zeros((B, D), dtype=np.float64)
    for c in range(NCORES):
        z = (results[c]["z"].reshape(HL, B).astype(np.float64) - pad_z
             + results[c]["pl"].reshape(HL, B).astype(np.float64))
        yp = results[c]["y"].astype(np.float64)  # (HL, B, D)
        y += (yp / z[:, :, None]).sum(axis=0)
    return y.astype(np.float32).reshape(B, 1, D)


def kernel(x, k_cache, v_cache, Wq, Wk, Wv, Wo, start_pos):
    start_pos = int(np.asarray(start_pos))
    S_eff = start_pos + 1
    in_maps = _prep_inputs(x, k_cache, v_cache, Wq, Wk, Wv, Wo, S_eff)
    nc = _PROGRAM_CACHE.get(S_eff)
    if nc is None:
        nc = build_program(S_eff)
        _PROGRAM_CACHE[S_eff] = nc
    res = run_bass_kernel_spmd(nc, in_maps, core_ids=list(range(NCORES)))
    return _combine(res.results, S_eff)


# revision 50
# speedup vs baseline: 1.0540x; 1.0540x over previous
"""Trainium2 Bass kernel for a cached-attention decode step (B=16, T=1, D=4096,
H=32, DK=128, S=2048), tensor-parallel over heads across 8 NeuronCores.

Sharding: each core owns 4 heads: column-sharded Wq/Wk/Wv (512 rows each),
the matching slices of the KV cache, and the matching 512 columns of Wo.

v3 design (vs the 196us baseline):
  - KV cache streamed entirely as fp8 e3m4 (33.5 MB/core); Wk/Wv fp8 x64
    (their quantization only touches the single new token); Wq/Wo fp16.
  - No k_new scatter into the cache: the host zeroes K column start_pos, the
    kernel exports pl = exp(q.k_new - SHIFT) and the host corrects
    Z = z - exp(-SHIFT) + pl. This removes the proj->attention serial
    dependency: scores depend only on q^T.
  - Monolithic weight DMAs (one transfer per tensor) to amortize the
    ~0.6us/transfer HWDGE fixed cost.
  - q-projection chains run first so attention starts ~13us into the body
    while wkv/wo still stream; y-projection runs inside the head loop.
  - kv is alone on the sync HWDGE ring (no head-of-line blocking); weights,
    x and all small stores ride the scalar/ACT ring.
  - Projections are weight-stationary matmuls producing q^T/k^T/v^T directly
    in [d, (h,b)] column layout (no PE transposes). All chains of a PSUM
    bank are opened by one zero matmul (first_mm clears the whole bank).

Math per core, per local head h and batch b:
    P = exp(scores/sqrt(DK) - EXP_SHIFT); zsum accumulated by the ACT engine
    AO_unnorm = P @ V (cache rows; the stale V row at start_pos is excluded
                       via the r_new row split) + pl * v_new
    y_h = AO_unnorm^T @ Wo_slice   (per-head, unnormalized)
Host divides y_h by Z per (head, batch), sums over heads and cores.
"""

from contextlib import ExitStack

import numpy as np

import concourse.bacc as bacc
import concourse.mybir as mybir
import concourse.tile as tile
from concourse.bass_utils import run_bass_kernel_spmd

B = 16          # batch
H = 32          # total heads
D = 4096        # model dim
DK = 128        # head dim
NCORES = 8
HL = H // NCORES            # 4 local heads per core
FL = HL * DK                # 512 local features per core
KT = D // 128               # 32 contraction tiles over D
F32 = mybir.dt.float32
F16 = mybir.dt.float16
F8 = mybir.dt.float8e3
AF = mybir.ActivationFunctionType

_PROGRAM_CACHE: dict = {}
_VARIANT = "full"  # "full" | "dma_only" | "compute_only" | "attn_only"
EXP_SHIFT = 2.0    # P = exp(score - EXP_SHIFT): cancels in host Z-normalization
SCALE = DK ** -0.5
WKV_SCALE = 64.0   # Wk/Wv pre-scaled x64 on host so fp8 e3m4 stays in range


def build_program(S_eff: int, repeat: int = 1, G: int = 2, kv_bufs: int = 8,
                  sc_bufs: int = 2, p_bufs: int = 8):
    """Emit the per-core Bass/Tile program (identical across all cores)."""
    NT = (S_eff + 127) // 128   # s-tiles incl. the partially-valid last tile
    S_pad = NT * 128
    r_new = (S_eff - 1) % 128   # row of the new token within the last s-tile
    SPV = S_pad + NT * DK       # fused fp8 K^T + V slab width per (h, b)
    assert B % G == 0
    HB = HL * B

    nc = bacc.Bacc("TRN2", num_devices=NCORES)
    xt = nc.declare_dram_parameter("xt", [128, KT, B], F16, isOutput=False)
    # wq is head-major so head h's q chain can start after 1/HL of the bytes
    wq_d = nc.declare_dram_parameter("wq", [128, HL, KT, DK], F16, isOutput=False)
    wkv_d = nc.declare_dram_parameter("wkv", [128, KT, 2 * FL], F8, isOutput=False)
    wo_d = nc.declare_dram_parameter("wo", [128, HL, D], F16, isOutput=False)
    kv_d = nc.declare_dram_parameter(
        "kv", [HL, B // G, 128, G, SPV], F8, isOutput=False
    )
    y_d = nc.declare_dram_parameter("y", [HL, B, D], F16, isOutput=True)
    z_d = nc.declare_dram_parameter("z", [1, HB], F32, isOutput=True)
    pl_d = nc.declare_dram_parameter("pl", [1, HB], F32, isOutput=True)

    with tile.TileContext(nc) as tc, ExitStack() as ctx:
        singles = ctx.enter_context(tc.tile_pool(name="singles", bufs=1))
        kvpool = ctx.enter_context(tc.tile_pool(name="kvp", bufs=kv_bufs))
        ppool = ctx.enter_context(tc.tile_pool(name="ppool", bufs=p_bufs))
        ysbp = ctx.enter_context(tc.tile_pool(name="ysbp", bufs=2))
        pqps = ctx.enter_context(tc.tile_pool(name="pq_ps", bufs=2, space="PSUM"))
        pkvps = ctx.enter_context(tc.tile_pool(name="pkv_ps", bufs=1, space="PSUM"))
        scps = ctx.enter_context(tc.tile_pool(name="sc_ps", bufs=sc_bufs, space="PSUM"))
        aops = ctx.enter_context(tc.tile_pool(name="ao_ps", bufs=1, space="PSUM"))
        mips = ctx.enter_context(tc.tile_pool(name="misc_ps", bufs=2, space="PSUM"))

        ones_col = singles.tile([128, 1], F32)
        nc.vector.memset(ones_col, 1.0)
        ones_row = singles.tile([1, 128], F32)
        nc.vector.memset(ones_row, 1.0)
        zero_q = singles.tile([1, HB], F32)
        nc.vector.memset(zero_q, 0.0)
        zero_kv = singles.tile([1, 2 * HB], F32)
        nc.vector.memset(zero_kv, 0.0)
        nbias = singles.tile([128, 1], F32)     # exp bias (-EXP_SHIFT)
        nc.vector.memset(nbias, -EXP_SHIFT)

        wq_sb = singles.tile([128, HL, KT, DK], F16)
        wkv_sb = singles.tile([128, KT, 2 * FL], F8)
        wo_sb = singles.tile([128, HL, D], F16)
        xt_sb = singles.tile([128, KT, B], F16)
        qT_sb = singles.tile([128, HB], F16)    # q^T columns per (h, b)
        if _VARIANT == "attn_only":
            nc.vector.memset(qT_sb, 0.01)
        kT_sb = singles.tile([128, HB], F32)    # k_new^T (true scale)
        vn_sb = singles.tile([128, HB], F32)    # v_new^T (true scale)
        vpl_sb = singles.tile([128, HB], F32)   # pl * v_new^T correction
        tmp_sb = singles.tile([128, HB], F32)
        pl_sb = singles.tile([1, HB], F32)      # exp(q . k_new - SHIFT)
        zsum_sb = singles.tile([128, HB], F32)
        ao_sb = singles.tile([128, HB], F16)
        z_sb = singles.tile([1, HB], F32)
        nc.vector.memset(z_sb, 0.0)
        nc.vector.memset(pl_sb, 0.0)
        if _VARIANT == "compute_only":
            kv_fix = singles.tile([128, SPV], F8)
            nc.vector.memset(kv_fix, 0.01)

        def body():
            dma_only = _VARIANT == "dma_only"
            attn_only = _VARIANT == "attn_only"

            if attn_only:
                for h in range(HL):
                    for g in range(B // G):
                        kv_grp = kvpool.tile([128, G, SPV], F8, tag="kv")
                        nc.sync.dma_start(out=kv_grp, in_=kv_d[h, g])
                        for j in range(G):
                            b = g * G + j
                            col = h * B + b
                            kv_sb = kv_grp[:, j, :]
                            sc_ps = scps.tile([128, NT], F32, tag="sc")
                            for t in range(NT):
                                nc.tensor.matmul(
                                    sc_ps[:, t:t + 1],
                                    lhsT=kv_sb[:, t * 128:(t + 1) * 128],
                                    rhs=qT_sb[:, col:col + 1],
                                    start=True, stop=True,
                                )
                            p_sb = ppool.tile([128, NT], F16, tag="p")
                            nc.scalar.activation(
                                out=p_sb, in_=sc_ps, func=AF.Exp,
                                bias=nbias[:128], scale=SCALE,
                                accum_out=zsum_sb[:, col:col + 1],
                            )
                            ao_ps = aops.tile([DK, B], F32, tag="ao")
                            for t in range(NT):
                                nc.tensor.matmul(
                                    ao_ps[:, b:b + 1],
                                    lhsT=kv_sb[:, S_pad + t * DK:S_pad + (t + 1) * DK],
                                    rhs=p_sb[:, t:t + 1],
                                    start=t == 0, stop=t == NT - 1,
                                )
                    nc.vector.tensor_copy(ao_sb[:, h * B:(h + 1) * B], ao_ps)
                z_ps = mips.tile([1, HB], F32, tag="mi")
                nc.tensor.matmul(z_ps, lhsT=ones_col, rhs=zsum_sb,
                                 start=True, stop=True)
                nc.vector.tensor_copy(z_sb, z_ps)
                nc.sync.dma_start(out=z_d[:, :], in_=z_sb)
                nc.scalar.dma_start(out=pl_d[:, :], in_=pl_sb)
                return

            # ---- loads: few big transfers, kv alone on the sync ring ----
            nc.scalar.dma_start(out=xt_sb, in_=xt[:, :, :])
            for h in range(HL):
                nc.scalar.dma_start(out=wq_sb[:, h], in_=wq_d[:, h])
            nc.scalar.dma_start(out=wkv_sb, in_=wkv_d[:, :, :])
            nc.scalar.dma_start(out=wo_sb, in_=wo_d[:, :, :])
            if dma_only:
                for h in range(HL):
                    for g in range(B // G):
                        kv_grp = kvpool.tile([128, G, SPV], F8, tag="kv",
                                             name="kv_grp")
                        nc.sync.dma_start(out=kv_grp, in_=kv_d[h, g])
                nc.sync.dma_start(out=z_d[:, :], in_=z_sb)
                nc.scalar.dma_start(out=pl_d[:, :], in_=pl_sb)
                return

            # ---- q-projection chains first (attention depends only on q^T).
            # Two PSUM banks, two heads each: head h's chain completes right
            # after its 1MB wq slab lands, and the bank's qT copy doesn't
            # wait on the other bank's chains (PE-W/DVE-R same-bank hazard).
            for hb in range(2):
                q_ps = pqps.tile([128, 2 * B], F32, tag="pq", name=f"q_ps{hb}")
                nc.tensor.matmul(q_ps, lhsT=ones_row, rhs=zero_q[:, :2 * B],
                                 start=True, stop=False)
                for hh in range(2):
                    h = 2 * hb + hh
                    for kt in range(KT):
                        nc.tensor.matmul(
                            q_ps[:, hh * B:(hh + 1) * B],
                            lhsT=wq_sb[:, h, kt, :],
                            rhs=xt_sb[:, kt, :],
                            start=False, stop=kt == KT - 1,
                        )
                nc.vector.tensor_copy(qT_sb[:, 2 * hb * B:(2 * hb + 2) * B],
                                      q_ps)

            def emit_kv_chains():
                # k/v-projection chains (separate PSUM bank). Emitted AFTER
                # head 0's score/PV stream so the PE FIFO never stalls
                # waiting for the wkv DMA while kv groups are ready.
                kv_ps = pkvps.tile([128, 2 * HB], F32, tag="pkv",
                                   name="kv_ps")
                nc.tensor.matmul(kv_ps, lhsT=ones_row, rhs=zero_kv,
                                 start=True, stop=False)
                for kt in range(KT):
                    for h in range(HL):
                        nc.tensor.matmul(
                            kv_ps[:, h * B:(h + 1) * B],
                            lhsT=wkv_sb[:, kt, h * DK:(h + 1) * DK],
                            rhs=xt_sb[:, kt, :],
                            start=False, stop=kt == KT - 1,
                        )
                        nc.tensor.matmul(
                            kv_ps[:, HB + h * B:HB + (h + 1) * B],
                            lhsT=wkv_sb[:, kt, FL + h * DK:FL + (h + 1) * DK],
                            rhs=xt_sb[:, kt, :],
                            start=False, stop=kt == KT - 1,
                        )
                nc.scalar.activation(out=kT_sb, in_=kv_ps[:, :HB],
                                     func=AF.Copy, scale=1.0 / WKV_SCALE)
                nc.scalar.activation(out=vn_sb, in_=kv_ps[:, HB:],
                                     func=AF.Copy, scale=1.0 / WKV_SCALE)
                # pl = exp(q . k_new - SHIFT) per (h, b); vpl = pl * v_new
                nc.vector.tensor_mul(tmp_sb, qT_sb, kT_sb)
                pl_ps = mips.tile([1, HB], F32, tag="mi", name="pl_ps")
                nc.tensor.matmul(pl_ps, lhsT=ones_col, rhs=tmp_sb,
                                 start=True, stop=True)
                nc.scalar.activation(out=pl_sb, in_=pl_ps, func=AF.Exp,
                                     bias=nbias[:1], scale=SCALE)
                plb_ps = mips.tile([128, HB], F32, tag="mi", name="plb_ps")
                nc.tensor.matmul(plb_ps, lhsT=ones_row, rhs=pl_sb,
                                 start=True, stop=True)
                nc.vector.tensor_mul(vpl_sb, vn_sb, plb_ps)

            emit_kv_chains()

            # ---- attention over the cache, head by head ----
            for h in range(HL):
                ao_ps = aops.tile([DK, B], F32, tag="ao")
                pending = []  # software-pipeline PV one bh behind scores

                def emit_pv(ent):
                    b_, p_sb_, kv_sb_ = ent
                    n_full = NT - 1
                    for t in range(n_full):
                        nc.tensor.matmul(
                            ao_ps[:, b_:b_ + 1],
                            lhsT=kv_sb_[:, S_pad + t * DK:S_pad + (t + 1) * DK],
                            rhs=p_sb_[:, t:t + 1],
                            start=t == 0,
                            stop=t == n_full - 1 and r_new == 0,
                        )
                    if r_new > 0:
                        nc.tensor.matmul(
                            ao_ps[:, b_:b_ + 1],
                            lhsT=kv_sb_[:r_new, S_pad + (NT - 1) * DK:S_pad + NT * DK],
                            rhs=p_sb_[:r_new, NT - 1:NT],
                            start=n_full == 0, stop=True,
                        )

                for g in range(B // G):
                    if _VARIANT == "compute_only":
                        kv_grp = None
                    else:
                        kv_grp = kvpool.tile([128, G, SPV], F8, tag="kv",
                                             name="kv_grp")
                        nc.sync.dma_start(out=kv_grp, in_=kv_d[h, g])
                    for j in range(G):
                        b = g * G + j
                        col = h * B + b
                        kv_sb = kv_fix if kv_grp is None else kv_grp[:, j, :]
                        sc_ps = scps.tile([128, NT], F32, tag="sc")
                        for t in range(NT):
                            nc.tensor.matmul(
                                sc_ps[:, t:t + 1],
                                lhsT=kv_sb[:, t * 128:(t + 1) * 128],
                                rhs=qT_sb[:, col:col + 1],
                                start=True, stop=True,
                            )
                        p_sb = ppool.tile([128, NT], F16, tag="p")
                        nc.scalar.activation(
                            out=p_sb, in_=sc_ps, func=AF.Exp,
                            bias=nbias[:128], scale=SCALE,
                            accum_out=zsum_sb[:, col:col + 1],
                        )
                        pending.append((b, p_sb, kv_sb))
                        if len(pending) == 7:
                            emit_pv(pending.pop(0))
                while pending:
                    emit_pv(pending.pop(0))

                # correction add during PSUM -> SBUF copy
                nc.vector.tensor_add(
                    ao_sb[:, h * B:(h + 1) * B], ao_ps,
                    vpl_sb[:, h * B:(h + 1) * B],
                )
                # per-head output projection (unnormalized), y on scalar ring
                y_sb = ysbp.tile([B, D], F16, tag="ysb")
                for oc in range(D // 512):
                    y_ps = mips.tile([B, 512], F32, tag="mi")
                    nc.tensor.matmul(
                        y_ps,
                        lhsT=ao_sb[:, h * B:(h + 1) * B],
                        rhs=wo_sb[:, h, oc * 512:(oc + 1) * 512],
                        start=True, stop=True,
                    )
                    nc.vector.tensor_copy(y_sb[:, oc * 512:(oc + 1) * 512], y_ps)
                nc.scalar.dma_start(out=y_d[h], in_=y_sb)

            # Z per (h, b): sum zsum over partitions via ones-matmul
            z_ps = mips.tile([1, HB], F32, tag="mi")
            nc.tensor.matmul(z_ps, lhsT=ones_col, rhs=zsum_sb,
                             start=True, stop=True)
            nc.vector.tensor_copy(z_sb, z_ps)
            nc.scalar.dma_start(out=z_d[:, :], in_=z_sb)
            nc.scalar.dma_start(out=pl_d[:, :], in_=pl_sb)

        if repeat == 1:
            body()
        else:
            # PE body is ~2500 instructions (>one IRAM block): hint the
            # back-edge so the branch target prefetches (~3-4us/iter saved)
            with tc.For_i(0, repeat, 1, hint_engines=(mybir.EngineType.PE,)):
                body()

    nc.compile()
    return nc


def _prep_inputs(x, k_cache, v_cache, Wq, Wk, Wv, Wo, S_eff, G=2):
    """Host-side sharding + layout prep. Returns per-core input dicts."""
    import ml_dtypes

    NT = (S_eff + 127) // 128
    S_pad = NT * 128
    SPV = S_pad + NT * DK
    f8 = ml_dtypes.float8_e3m4

    x2 = np.asarray(x, dtype=np.float32).reshape(B, D)
    xt_tiled = np.ascontiguousarray(
        x2.T.reshape(KT, 128, B).transpose(1, 0, 2).astype(np.float16)
    )  # [128, KT, B]

    k_cache = np.asarray(k_cache, dtype=np.float32)
    v_cache = np.asarray(v_cache, dtype=np.float32)

    # fused fp8 slab: K^T cols [0, S_pad), V tiles [S_pad, S_pad + NT*DK).
    # K column S_eff-1 (the new token) is left ZERO: its score is then 0 and
    # contributes exp(-EXP_SHIFT) to zsum, corrected on the host with pl.
    kv_all = np.zeros((H, B, 128, SPV), dtype=f8)
    kv_all[:, :, :, :S_eff - 1] = (
        k_cache[:, :, :S_eff - 1, :].transpose(1, 0, 3, 2)
    )
    v_src = np.zeros((H, B, NT * 128, DK), dtype=f8)
    v_src[:, :, :S_eff] = v_cache[:, :, :S_eff].transpose(1, 0, 2, 3)
    kv_all[:, :, :, S_pad:] = (
        v_src.reshape(H, B, NT, 128, DK)
        .transpose(0, 1, 3, 2, 4)
        .reshape(H, B, 128, NT * DK)
    )
    del v_src

    Wq = np.asarray(Wq, dtype=np.float32)
    Wk = np.asarray(Wk, dtype=np.float32)
    Wv = np.asarray(Wv, dtype=np.float32)
    Wo = np.asarray(Wo, dtype=np.float32)

    in_maps = []
    for c in range(NCORES):
        rows = slice(c * FL, (c + 1) * FL)
        # head-major: [128, HL, KT, DK], element [p, h, kt, dk] =
        # Wq^T[kt*128+p, h*DK+dk] restricted to this core's rows
        wq_tiled = np.ascontiguousarray(
            Wq[rows].T.reshape(KT, 128, HL, DK)
            .transpose(1, 2, 0, 3).astype(np.float16)
        )
        wkv_c = np.concatenate(
            [Wk[rows].T * WKV_SCALE, Wv[rows].T * WKV_SCALE], axis=1
        )  # (D, 2*FL)
        wkv_tiled = np.ascontiguousarray(
            wkv_c.reshape(KT, 128, 2 * FL).transpose(1, 0, 2).astype(f8)
        )
        wo_c = Wo[:, rows].T  # (FL, D)
        wo_tiled = np.ascontiguousarray(
            wo_c.reshape(HL, 128, D).transpose(1, 0, 2).astype(np.float16)
        )
        # kv: [HL, B, 128, SPV] -> grouped contiguous [HL, B//G, 128, G, SPV]
        kv_c = kv_all[c * HL:(c + 1) * HL]
        kv_c = np.ascontiguousarray(
            kv_c.reshape(HL, B // G, G, 128, SPV).transpose(0, 1, 3, 2, 4)
        )
        in_maps.append({
            "xt": xt_tiled,
            "wq": wq_tiled,
            "wkv": wkv_tiled,
            "wo": wo_tiled,
            "kv": kv_c,
        })
    return in_maps


def _combine(results, S_eff):
    """Host-side unshard: divide per-head partials by Z, sum everything."""
    NT = (S_eff + 127) // 128
    # zero cache columns (padding + the zeroed new-token column) each
    # contribute exp(0 - EXP_SHIFT) to zsum; the true new-token term is pl.
    pad_z = (NT * 128 - S_eff + 1) * np.exp(-EXP_SHIFT)
    y = np.zeros((B, D), dtype=np.float64)
    for c in range(NCORES):
        z = (results[c]["z"].reshape(HL, B).astype(np.float64) - pad_z
             + results[c]["pl"].reshape(HL, B).astype(np.float64))
        yp = results[c]["y"].astype(np.float64)  # (HL, B, D)
        y += (yp / z[:, :, None]).sum(axis=0)
    return y.astype(np.float32).reshape(B, 1, D)


def kernel(x, k_cache, v_cache, Wq, Wk, Wv, Wo, start_pos):
    start_pos = int(np.asarray(start_pos))
    S_eff = start_pos + 1
    in_maps = _prep_inputs(x, k_cache, v_cache, Wq, Wk, Wv, Wo, S_eff)
    nc = _PROGRAM_CACHE.get(S_eff)
    if nc is None:
        nc = build_program(S_eff)
        _PROGRAM_CACHE[S_eff] = nc
    res = run_bass_kernel_spmd(nc, in_maps, core_ids=list(range(NCORES)))
    return _combine(res.results, S_eff)


# revision 51
# speedup vs baseline: 1.0775x; 1.0223x over previous
"""Trainium2 Bass kernel for a cached-attention decode step (B=16, T=1, D=4096,
H=32, DK=128, S=2048), tensor-parallel over heads across 8 NeuronCores.

Sharding: each core owns 4 heads: column-sharded Wq/Wk/Wv (512 rows each),
the matching slices of the KV cache, and the matching 512 columns of Wo.

v3 design (vs the 196us baseline):
  - KV cache streamed entirely as fp8 e3m4 (33.5 MB/core); Wk/Wv fp8 x64
    (their quantization only touches the single new token); Wq/Wo fp16.
  - No k_new scatter into the cache: the host zeroes K column start_pos, the
    kernel exports pl = exp(q.k_new - SHIFT) and the host corrects
    Z = z - exp(-SHIFT) + pl. This removes the proj->attention serial
    dependency: scores depend only on q^T.
  - Monolithic weight DMAs (one transfer per tensor) to amortize the
    ~0.6us/transfer HWDGE fixed cost.
  - q-projection chains run first so attention starts ~13us into the body
    while wkv/wo still stream; y-projection runs inside the head loop.
  - kv is alone on the sync HWDGE ring (no head-of-line blocking); weights,
    x and all small stores ride the scalar/ACT ring.
  - Projections are weight-stationary matmuls producing q^T/k^T/v^T directly
    in [d, (h,b)] column layout (no PE transposes). All chains of a PSUM
    bank are opened by one zero matmul (first_mm clears the whole bank).

Math per core, per local head h and batch b:
    P = exp(scores/sqrt(DK) - EXP_SHIFT); zsum accumulated by the ACT engine
    AO_unnorm = P @ V (cache rows; the stale V row at start_pos is excluded
                       via the r_new row split) + pl * v_new
    y_h = AO_unnorm^T @ Wo_slice   (per-head, unnormalized)
Host divides y_h by Z per (head, batch), sums over heads and cores.
"""

from contextlib import ExitStack

import numpy as np

import concourse.bacc as bacc
import concourse.mybir as mybir
import concourse.tile as tile
from concourse.bass_utils import run_bass_kernel_spmd

B = 16          # batch
H = 32          # total heads
D = 4096        # model dim
DK = 128        # head dim
NCORES = 8
HL = H // NCORES            # 4 local heads per core
FL = HL * DK                # 512 local features per core
KT = D // 128               # 32 contraction tiles over D
F32 = mybir.dt.float32
F16 = mybir.dt.float16
F8 = mybir.dt.float8e3
AF = mybir.ActivationFunctionType

_PROGRAM_CACHE: dict = {}
_VARIANT = "full"  # "full" | "dma_only" | "compute_only" | "attn_only"
EXP_SHIFT = 2.0    # P = exp(score - EXP_SHIFT): cancels in host Z-normalization
SCALE = DK ** -0.5
WKV_SCALE = 64.0   # Wk/Wv pre-scaled x64 on host so fp8 e3m4 stays in range


def build_program(S_eff: int, repeat: int = 1, G: int = 2, kv_bufs: int = 8,
                  sc_bufs: int = 2, p_bufs: int = 11):
    """Emit the per-core Bass/Tile program (identical across all cores)."""
    NT = (S_eff + 127) // 128   # s-tiles incl. the partially-valid last tile
    S_pad = NT * 128
    r_new = (S_eff - 1) % 128   # row of the new token within the last s-tile
    SPV = S_pad + NT * DK       # fused fp8 K^T + V slab width per (h, b)
    assert B % G == 0
    HB = HL * B

    nc = bacc.Bacc("TRN2", num_devices=NCORES)
    xt = nc.declare_dram_parameter("xt", [128, KT, B], F16, isOutput=False)
    # wq is head-major so head h's q chain can start after 1/HL of the bytes
    wq_d = nc.declare_dram_parameter("wq", [128, HL, KT, DK], F16, isOutput=False)
    wkv_d = nc.declare_dram_parameter("wkv", [128, KT, 2 * FL], F8, isOutput=False)
    wo_d = nc.declare_dram_parameter("wo", [128, HL, D], F16, isOutput=False)
    kv_d = nc.declare_dram_parameter(
        "kv", [HL, B // G, 128, G, SPV], F8, isOutput=False
    )
    y_d = nc.declare_dram_parameter("y", [HL, B, D], F16, isOutput=True)
    z_d = nc.declare_dram_parameter("z", [1, HB], F32, isOutput=True)
    pl_d = nc.declare_dram_parameter("pl", [1, HB], F32, isOutput=True)

    with tile.TileContext(nc) as tc, ExitStack() as ctx:
        singles = ctx.enter_context(tc.tile_pool(name="singles", bufs=1))
        kvpool = ctx.enter_context(tc.tile_pool(name="kvp", bufs=kv_bufs))
        ppool = ctx.enter_context(tc.tile_pool(name="ppool", bufs=p_bufs))
        ysbp = ctx.enter_context(tc.tile_pool(name="ysbp", bufs=2))
        pqps = ctx.enter_context(tc.tile_pool(name="pq_ps", bufs=2, space="PSUM"))
        pkvps = ctx.enter_context(tc.tile_pool(name="pkv_ps", bufs=1, space="PSUM"))
        scps = ctx.enter_context(tc.tile_pool(name="sc_ps", bufs=sc_bufs, space="PSUM"))
        aops = ctx.enter_context(tc.tile_pool(name="ao_ps", bufs=1, space="PSUM"))
        mips = ctx.enter_context(tc.tile_pool(name="misc_ps", bufs=2, space="PSUM"))

        ones_col = singles.tile([128, 1], F32)
        nc.vector.memset(ones_col, 1.0)
        ones_row = singles.tile([1, 128], F32)
        nc.vector.memset(ones_row, 1.0)
        zero_q = singles.tile([1, HB], F32)
        nc.vector.memset(zero_q, 0.0)
        zero_kv = singles.tile([1, 2 * HB], F32)
        nc.vector.memset(zero_kv, 0.0)
        nbias = singles.tile([128, 1], F32)     # exp bias (-EXP_SHIFT)
        nc.vector.memset(nbias, -EXP_SHIFT)

        wq_sb = singles.tile([128, HL, KT, DK], F16)
        wkv_sb = singles.tile([128, KT, 2 * FL], F8)
        wo_sb = singles.tile([128, HL, D], F16)
        xt_sb = singles.tile([128, KT, B], F16)
        qT_sb = singles.tile([128, HB], F16)    # q^T columns per (h, b)
        if _VARIANT == "attn_only":
            nc.vector.memset(qT_sb, 0.01)
        kT_sb = singles.tile([128, HB], F32)    # k_new^T (true scale)
        vn_sb = singles.tile([128, HB], F32)    # v_new^T (true scale)
        vpl_sb = singles.tile([128, HB], F32)   # pl * v_new^T correction
        tmp_sb = singles.tile([128, HB], F32)
        pl_sb = singles.tile([1, HB], F32)      # exp(q . k_new - SHIFT)
        zsum_sb = singles.tile([128, HB], F32)
        ao_sb = singles.tile([128, HB], F16)
        z_sb = singles.tile([1, HB], F32)
        nc.vector.memset(z_sb, 0.0)
        nc.vector.memset(pl_sb, 0.0)
        if _VARIANT == "compute_only":
            kv_fix = singles.tile([128, SPV], F8)
            nc.vector.memset(kv_fix, 0.01)

        def body():
            dma_only = _VARIANT == "dma_only"
            attn_only = _VARIANT == "attn_only"

            if attn_only:
                for h in range(HL):
                    for g in range(B // G):
                        kv_grp = kvpool.tile([128, G, SPV], F8, tag="kv")
                        nc.sync.dma_start(out=kv_grp, in_=kv_d[h, g])
                        for j in range(G):
                            b = g * G + j
                            col = h * B + b
                            kv_sb = kv_grp[:, j, :]
                            sc_ps = scps.tile([128, NT], F32, tag="sc")
                            for t in range(NT):
                                nc.tensor.matmul(
                                    sc_ps[:, t:t + 1],
                                    lhsT=kv_sb[:, t * 128:(t + 1) * 128],
                                    rhs=qT_sb[:, col:col + 1],
                                    start=True, stop=True,
                                )
                            p_sb = ppool.tile([128, NT], F16, tag="p")
                            nc.scalar.activation(
                                out=p_sb, in_=sc_ps, func=AF.Exp,
                                bias=nbias[:128], scale=SCALE,
                                accum_out=zsum_sb[:, col:col + 1],
                            )
                            ao_ps = aops.tile([DK, B], F32, tag="ao")
                            for t in range(NT):
                                nc.tensor.matmul(
                                    ao_ps[:, b:b + 1],
                                    lhsT=kv_sb[:, S_pad + t * DK:S_pad + (t + 1) * DK],
                                    rhs=p_sb[:, t:t + 1],
                                    start=t == 0, stop=t == NT - 1,
                                )
                    nc.vector.tensor_copy(ao_sb[:, h * B:(h + 1) * B], ao_ps)
                z_ps = mips.tile([1, HB], F32, tag="mi")
                nc.tensor.matmul(z_ps, lhsT=ones_col, rhs=zsum_sb,
                                 start=True, stop=True)
                nc.vector.tensor_copy(z_sb, z_ps)
                nc.sync.dma_start(out=z_d[:, :], in_=z_sb)
                nc.scalar.dma_start(out=pl_d[:, :], in_=pl_sb)
                return

            # ---- loads: few big transfers, kv alone on the sync ring ----
            nc.scalar.dma_start(out=xt_sb, in_=xt[:, :, :])
            for h in range(HL):
                nc.scalar.dma_start(out=wq_sb[:, h], in_=wq_d[:, h])
            nc.scalar.dma_start(out=wkv_sb, in_=wkv_d[:, :, :])
            nc.scalar.dma_start(out=wo_sb, in_=wo_d[:, :, :])
            if dma_only:
                for h in range(HL):
                    for g in range(B // G):
                        kv_grp = kvpool.tile([128, G, SPV], F8, tag="kv",
                                             name="kv_grp")
                        nc.sync.dma_start(out=kv_grp, in_=kv_d[h, g])
                nc.sync.dma_start(out=z_d[:, :], in_=z_sb)
                nc.scalar.dma_start(out=pl_d[:, :], in_=pl_sb)
                return

            # ---- q-projection chains first (attention depends only on q^T).
            # Two PSUM banks, two heads each: head h's chain completes right
            # after its 1MB wq slab lands, and the bank's qT copy doesn't
            # wait on the other bank's chains (PE-W/DVE-R same-bank hazard).
            for hb in range(2):
                q_ps = pqps.tile([128, 2 * B], F32, tag="pq", name=f"q_ps{hb}")
                nc.tensor.matmul(q_ps, lhsT=ones_row, rhs=zero_q[:, :2 * B],
                                 start=True, stop=False)
                for hh in range(2):
                    h = 2 * hb + hh
                    for kt in range(KT):
                        nc.tensor.matmul(
                            q_ps[:, hh * B:(hh + 1) * B],
                            lhsT=wq_sb[:, h, kt, :],
                            rhs=xt_sb[:, kt, :],
                            start=False, stop=kt == KT - 1,
                        )
                nc.vector.tensor_copy(qT_sb[:, 2 * hb * B:(2 * hb + 2) * B],
                                      q_ps)

            def emit_kv_chains():
                # k/v-projection chains (separate PSUM bank). Emitted AFTER
                # head 0's score/PV stream so the PE FIFO never stalls
                # waiting for the wkv DMA while kv groups are ready.
                kv_ps = pkvps.tile([128, 2 * HB], F32, tag="pkv",
                                   name="kv_ps")
                nc.tensor.matmul(kv_ps, lhsT=ones_row, rhs=zero_kv,
                                 start=True, stop=False)
                for kt in range(KT):
                    for h in range(HL):
                        nc.tensor.matmul(
                            kv_ps[:, h * B:(h + 1) * B],
                            lhsT=wkv_sb[:, kt, h * DK:(h + 1) * DK],
                            rhs=xt_sb[:, kt, :],
                            start=False, stop=kt == KT - 1,
                        )
                        nc.tensor.matmul(
                            kv_ps[:, HB + h * B:HB + (h + 1) * B],
                            lhsT=wkv_sb[:, kt, FL + h * DK:FL + (h + 1) * DK],
                            rhs=xt_sb[:, kt, :],
                            start=False, stop=kt == KT - 1,
                        )
                nc.scalar.activation(out=kT_sb, in_=kv_ps[:, :HB],
                                     func=AF.Copy, scale=1.0 / WKV_SCALE)
                nc.scalar.activation(out=vn_sb, in_=kv_ps[:, HB:],
                                     func=AF.Copy, scale=1.0 / WKV_SCALE)
                # pl = exp(q . k_new - SHIFT) per (h, b); vpl = pl * v_new
                nc.vector.tensor_mul(tmp_sb, qT_sb, kT_sb)
                pl_ps = mips.tile([1, HB], F32, tag="mi", name="pl_ps")
                nc.tensor.matmul(pl_ps, lhsT=ones_col, rhs=tmp_sb,
                                 start=True, stop=True)
                nc.scalar.activation(out=pl_sb, in_=pl_ps, func=AF.Exp,
                                     bias=nbias[:1], scale=SCALE)
                plb_ps = mips.tile([128, HB], F32, tag="mi", name="plb_ps")
                nc.tensor.matmul(plb_ps, lhsT=ones_row, rhs=pl_sb,
                                 start=True, stop=True)
                nc.vector.tensor_mul(vpl_sb, vn_sb, plb_ps)

            emit_kv_chains()

            # ---- attention over the cache, head by head ----
            for h in range(HL):
                ao_ps = aops.tile([DK, B], F32, tag="ao")
                pending = []  # software-pipeline PV one bh behind scores

                def emit_pv(ent):
                    b_, p_sb_, kv_sb_ = ent
                    n_full = NT - 1
                    for t in range(n_full):
                        nc.tensor.matmul(
                            ao_ps[:, b_:b_ + 1],
                            lhsT=kv_sb_[:, S_pad + t * DK:S_pad + (t + 1) * DK],
                            rhs=p_sb_[:, t:t + 1],
                            start=t == 0,
                            stop=t == n_full - 1 and r_new == 0,
                        )
                    if r_new > 0:
                        nc.tensor.matmul(
                            ao_ps[:, b_:b_ + 1],
                            lhsT=kv_sb_[:r_new, S_pad + (NT - 1) * DK:S_pad + NT * DK],
                            rhs=p_sb_[:r_new, NT - 1:NT],
                            start=n_full == 0, stop=True,
                        )

                for g in range(B // G):
                    if _VARIANT == "compute_only":
                        kv_grp = None
                    else:
                        kv_grp = kvpool.tile([128, G, SPV], F8, tag="kv",
                                             name="kv_grp")
                        nc.sync.dma_start(out=kv_grp, in_=kv_d[h, g])
                    for j in range(G):
                        b = g * G + j
                        col = h * B + b
                        kv_sb = kv_fix if kv_grp is None else kv_grp[:, j, :]
                        sc_ps = scps.tile([128, NT], F32, tag="sc")
                        for t in range(NT):
                            nc.tensor.matmul(
                                sc_ps[:, t:t + 1],
                                lhsT=kv_sb[:, t * 128:(t + 1) * 128],
                                rhs=qT_sb[:, col:col + 1],
                                start=True, stop=True,
                            )
                        p_sb = ppool.tile([128, NT], F16, tag="p")
                        nc.scalar.activation(
                            out=p_sb, in_=sc_ps, func=AF.Exp,
                            bias=nbias[:128], scale=SCALE,
                            accum_out=zsum_sb[:, col:col + 1],
                        )
                        pending.append((b, p_sb, kv_sb))
                        if len(pending) == 10:
                            emit_pv(pending.pop(0))
                while pending:
                    emit_pv(pending.pop(0))

                # correction add during PSUM -> SBUF copy
                nc.vector.tensor_add(
                    ao_sb[:, h * B:(h + 1) * B], ao_ps,
                    vpl_sb[:, h * B:(h + 1) * B],
                )
                # per-head output projection (unnormalized), y on scalar ring
                y_sb = ysbp.tile([B, D], F16, tag="ysb")
                for oc in range(D // 512):
                    y_ps = mips.tile([B, 512], F32, tag="mi")
                    nc.tensor.matmul(
                        y_ps,
                        lhsT=ao_sb[:, h * B:(h + 1) * B],
                        rhs=wo_sb[:, h, oc * 512:(oc + 1) * 512],
                        start=True, stop=True,
                    )
                    nc.vector.tensor_copy(y_sb[:, oc * 512:(oc + 1) * 512], y_ps)
                nc.scalar.dma_start(out=y_d[h], in_=y_sb)

            # Z per (h, b): sum zsum over partitions via ones-matmul
            z_ps = mips.tile([1, HB], F32, tag="mi")
            nc.tensor.matmul(z_ps, lhsT=ones_col, rhs=zsum_sb,
                             start=True, stop=True)
            nc.vector.tensor_copy(z_sb, z_ps)
            nc.scalar.dma_start(out=z_d[:, :], in_=z_sb)
            nc.scalar.dma_start(out=pl_d[:, :], in_=pl_sb)

        if repeat == 1:
            body()
        else:
            # PE body is ~2500 instructions (>one IRAM block): hint the
            # back-edge so the branch target prefetches (~3-4us/iter saved)
            with tc.For_i(0, repeat, 1, hint_engines=(mybir.EngineType.PE,)):
                body()

    nc.compile()
    return nc


def _prep_inputs(x, k_cache, v_cache, Wq, Wk, Wv, Wo, S_eff, G=2):
    """Host-side sharding + layout prep. Returns per-core input dicts."""
    import ml_dtypes

    NT = (S_eff + 127) // 128
    S_pad = NT * 128
    SPV = S_pad + NT * DK
    f8 = ml_dtypes.float8_e3m4

    x2 = np.asarray(x, dtype=np.float32).reshape(B, D)
    xt_tiled = np.ascontiguousarray(
        x2.T.reshape(KT, 128, B).transpose(1, 0, 2).astype(np.float16)
    )  # [128, KT, B]

    k_cache = np.asarray(k_cache, dtype=np.float32)
    v_cache = np.asarray(v_cache, dtype=np.float32)

    # fused fp8 slab: K^T cols [0, S_pad), V tiles [S_pad, S_pad + NT*DK).
    # K column S_eff-1 (the new token) is left ZERO: its score is then 0 and
    # contributes exp(-EXP_SHIFT) to zsum, corrected on the host with pl.
    kv_all = np.zeros((H, B, 128, SPV), dtype=f8)
    kv_all[:, :, :, :S_eff - 1] = (
        k_cache[:, :, :S_eff - 1, :].transpose(1, 0, 3, 2)
    )
    v_src = np.zeros((H, B, NT * 128, DK), dtype=f8)
    v_src[:, :, :S_eff] = v_cache[:, :, :S_eff].transpose(1, 0, 2, 3)
    kv_all[:, :, :, S_pad:] = (
        v_src.reshape(H, B, NT, 128, DK)
        .transpose(0, 1, 3, 2, 4)
        .reshape(H, B, 128, NT * DK)
    )
    del v_src

    Wq = np.asarray(Wq, dtype=np.float32)
    Wk = np.asarray(Wk, dtype=np.float32)
    Wv = np.asarray(Wv, dtype=np.float32)
    Wo = np.asarray(Wo, dtype=np.float32)

    in_maps = []
    for c in range(NCORES):
        rows = slice(c * FL, (c + 1) * FL)
        # head-major: [128, HL, KT, DK], element [p, h, kt, dk] =
        # Wq^T[kt*128+p, h*DK+dk] restricted to this core's rows
        wq_tiled = np.ascontiguousarray(
            Wq[rows].T.reshape(KT, 128, HL, DK)
            .transpose(1, 2, 0, 3).astype(np.float16)
        )
        wkv_c = np.concatenate(
            [Wk[rows].T * WKV_SCALE, Wv[rows].T * WKV_SCALE], axis=1
        )  # (D, 2*FL)
        wkv_tiled = np.ascontiguousarray(
            wkv_c.reshape(KT, 128, 2 * FL).transpose(1, 0, 2).astype(f8)
        )
        wo_c = Wo[:, rows].T  # (FL, D)
        wo_tiled = np.ascontiguousarray(
            wo_c.reshape(HL, 128, D).transpose(1, 0, 2).astype(np.float16)
        )
        # kv: [HL, B, 128, SPV] -> grouped contiguous [HL, B//G, 128, G, SPV]
        kv_c = kv_all[c * HL:(c + 1) * HL]
        kv_c = np.ascontiguousarray(
            kv_c.reshape(HL, B // G, G, 128, SPV).transpose(0, 1, 3, 2, 4)
        )
        in_maps.append({
            "xt": xt_tiled,
            "wq": wq_tiled,
            "wkv": wkv_tiled,
            "wo": wo_tiled,
            "kv": kv_c,
        })
    return in_maps


def _combine(results, S_eff):
    """Host-side unshard: divide per-head partials by Z, sum everything."""
    NT = (S_eff + 127) // 128
    # zero cache columns (padding + the zeroed new-token column) each
    # contribute exp(0 - EXP_SHIFT) to zsum; the true new-token term is pl.
    pad_z = (NT * 128 - S_eff + 1) * np.exp(-EXP_SHIFT)
    y = np.zeros((B, D), dtype=np.float64)
    for c in range(NCORES):
        z = (results[c]["z"].reshape(HL, B).astype(np.float64) - pad_z
             + results[c]["pl"].reshape(HL, B).astype(np.float64))
        yp = results[c]["y"].astype(np.float64)  # (HL, B, D)
        y += (yp / z[:, :, None]).sum(axis=0)
    return y.astype(np.float32).reshape(B, 1, D)


def kernel(x, k_cache, v_cache, Wq, Wk, Wv, Wo, start_pos):
    start_pos = int(np.asarray(start_pos))
    S_eff = start_pos + 1
    in_maps = _prep_inputs(x, k_cache, v_cache, Wq, Wk, Wv, Wo, S_eff)
    nc = _PROGRAM_CACHE.get(S_eff)
    if nc is None:
        nc = build_program(S_eff)
        _PROGRAM_CACHE[S_eff] = nc
    res = run_bass_kernel_spmd(nc, in_maps, core_ids=list(range(NCORES)))
    return _combine(res.results, S_eff)
